# revision 15
# baseline (speedup 1.0000x reference)
"""TRN2 Bass/Tile kernel for NeoTCNAttention (talking-heads attention with
ALiBi + persistent memory), SPMD over 8 NeuronCores.

Sharding: data-parallel over batch N=4 x 2 halves of the query axis
(each core: one batch element, 1024 query positions, full keys/values).
No collectives: every core computes a disjoint slab of the output.

Per-core pipeline (all matmuls are standard full-array ops):
  - activations arrive transposed/fp16 (x^T pair tiles [128=(2 heads x 64
    dims), seq]); Q/K/V projections use host-built block-diag(W^T, W^T)
    weights, one matmul per 512-wide chunk.
  - W_pre (pre-softmax talking heads) is folded into the score matmuls:
    q~[(h,d), (g, qi)] = W_pre[g, h] * q'^T, so for each output bank
    (4 g-heads x 32 q rows) the scores+mix are 4 accumulated pair matmuls
    plus one ALiBi matmul against a distance table (ALiBi slopes folded
    through W_pre into per-g scalars c_g).
  - exp runs on the scalar engine straight out of PSUM with accumulated
    row sums; softmax normalization is folded into the post-softmax
    talking-heads matrix (per-row 1/sum scaling).
  - post-softmax talking heads run as transposing matmuls (exp tile as
    the stationary operand) producing A'^T [k, (g, q)] directly, which
    feeds A@V with v' columns as stationary weights.
  - fc_out accumulates 8 K=64 matmuls with the bias folded in as a
    rank-1 (ones x fc_b) matmul.
"""

import math

import numpy as np

# ---- problem constants (hardcoded per spec) ----
N_BATCH = 4
SEQ = 2048
EMBED = 512
HEADS = 8
HD = 64
NPERS = 16
KT = SEQ + NPERS  # 2064
ALIBI_ALPHA = 1.25
START_I = 1
N_CORES = 8
SQ = 1024  # query positions per core
SCALE = 1.0 / math.sqrt(EMBED)

QB = 32            # query positions per block (bank rows = 4 g x 32 qi)
NQB = SQ // QB     # 32 blocks
QG = 256           # query positions per A@V group
NQG = SQ // QG     # 4 groups
QBPG = QG // QB    # 8 blocks per group
KC = 512           # k chunk width for scores/exp
NKC = SEQ // KC    # 4 (persistent 16 handled separately)
NKJ = SEQ // 128   # 16 mix2/A@V k-subchunks (+1 partial of NPERS)
DIST_W = SEQ + KT  # 4112 distance-table width

_CACHED = {}
_last_in_maps = None


def _host_consts(Wv, Wk, Wq, W_pre, W_post, p_keys, p_values, fc_w, fc_b):
    """Derived constant tensors shipped to the device (layout prep only)."""
    f16 = np.float16
    Z = np.zeros((HD, HD), np.float32)
    c = {}
    c["wq2"] = np.block([[Wq.T, Z], [Z, Wq.T]]).astype(f16)  # [128, 128]
    c["wk2"] = np.block([[Wk.T, Z], [Z, Wk.T]]).astype(f16)
    c["wv2"] = np.block([[Wv.T, Z], [Z, Wv.T]]).astype(f16)
    c["pkT"] = np.ascontiguousarray(p_keys[:, 0, :].T).astype(f16)   # [64, 16]
    c["pv"] = np.ascontiguousarray(p_values[:, 0, :]).astype(f16)    # [16, 64]

    # W_pre scaling columns for the q~ build: col 4*g + p scales pair p's
    # rows (head 2p in partitions 0-63, head 2p+1 in 64-127) for out-head g.
    wpcol = np.zeros((128, HEADS * 4), np.float32)
    for g in range(HEADS):
        for p in range(4):
            wpcol[0:HD, 4 * g + p] = W_pre[g, 2 * p]
            wpcol[HD:128, 4 * g + p] = W_pre[g, 2 * p + 1]
    c["wpcol"] = wpcol

    slopes = 2.0 ** (
        -ALIBI_ALPHA * (np.arange(1, HEADS + 1, dtype=np.float64) + START_I)
    )
    cg_vec = -(W_pre.astype(np.float64) @ slopes)  # per-out-head slope mix

    cgcol = np.zeros((128, 2), np.float32)
    for oi in range(2):
        for ga in range(4):
            cgcol[32 * ga : 32 * (ga + 1), oi] = cg_vec[4 * oi + ga]
    c["cgcol"] = cgcol

    idx = np.arange(QB)
    for ob, gofs in (("A", 0), ("B", 4)):
        w2p = np.zeros((128, 256), np.float32)
        for ga in range(4):
            for g2 in range(HEADS):
                w2p[32 * ga + idx, 32 * g2 + idx] = W_post[g2, gofs + ga]
        c[f"w2p{ob}"] = w2p.astype(np.float32)

    c["fcwT"] = np.ascontiguousarray(fc_w.T).astype(f16)  # [512, 512] rows=(g,d)
    c["fcb"] = fc_b.reshape(1, EMBED).astype(f16)
    return c


def _dist_table(qbase: int) -> np.ndarray:
    """T[32*ga + qi, u] = |qbase + qi - (u - SEQ)| as fp16 (rows replicated
    over the 4 out-heads of a bank; exact for ints <= 2048)."""
    u = np.arange(DIST_W)
    qi = np.arange(QB)
    t = np.abs(qbase + qi[:, None] - (u[None, :] - SEQ)).astype(np.float16)
    return np.tile(t, (4, 1))


def build_bass():
    import concourse.mybir as mybir
    import concourse.tile as tile
    from concourse import bacc
    from contextlib import ExitStack

    f32 = mybir.dt.float32
    f16 = mybir.dt.float16
    bf16 = mybir.dt.bfloat16
    EXP = mybir.ActivationFunctionType.Exp
    X = mybir.AxisListType.X

    nc = bacc.Bacc(
        "TRN2", target_bir_lowering=False, debug=False, num_devices=N_CORES
    )

    # ---- DRAM I/O ----
    qT_d = nc.dram_tensor("qT16", [EMBED, SQ], f16, kind="ExternalInput").ap()
    kT_d = nc.dram_tensor("kT16", [EMBED, SEQ], f16, kind="ExternalInput").ap()
    vT_d = nc.dram_tensor("vT16", [EMBED, SEQ], f16, kind="ExternalInput").ap()
    wq2_d = nc.dram_tensor("wq2", [128, 128], f16, kind="ExternalInput").ap()
    wk2_d = nc.dram_tensor("wk2", [128, 128], f16, kind="ExternalInput").ap()
    wv2_d = nc.dram_tensor("wv2", [128, 128], f16, kind="ExternalInput").ap()
    pkT_d = nc.dram_tensor("pkT", [HD, NPERS], f16, kind="ExternalInput").ap()
    pv_d = nc.dram_tensor("pv", [NPERS, HD], f16, kind="ExternalInput").ap()
    wpcol_d = nc.dram_tensor(
        "wpcol", [128, HEADS * 4], f32, kind="ExternalInput"
    ).ap()
    cgcol_d = nc.dram_tensor("cgcol", [128, 2], f32, kind="ExternalInput").ap()
    w2p_d = {
        nm: nc.dram_tensor(nm, [128, 256], f32, kind="ExternalInput").ap()
        for nm in ("w2pA", "w2pB")
    }
    dist_d = nc.dram_tensor("dist", [128, DIST_W], f16, kind="ExternalInput").ap()
    fcwT_d = nc.dram_tensor("fcwT", [EMBED, EMBED], f16, kind="ExternalInput").ap()
    fcb_d = nc.dram_tensor("fcb", [1, EMBED], f16, kind="ExternalInput").ap()
    out_d = nc.dram_tensor("out", [SQ, EMBED], f32, kind="ExternalOutput").ap()

    def ecopy(eng, out, in_):
        if eng is nc.scalar:
            eng.copy(out, in_)
        else:
            eng.tensor_copy(out, in_)

    with tile.TileContext(nc) as tc, ExitStack() as ctx:
        const_pool = ctx.enter_context(tc.tile_pool(name="const", bufs=1))
        xT_pool = ctx.enter_context(tc.tile_pool(name="xT", bufs=1))
        vs_pool = ctx.enter_context(tc.tile_pool(name="vs", bufs=1))
        qt_pool = ctx.enter_context(tc.tile_pool(name="qt", bufs=1))
        xraw_pool = ctx.enter_context(tc.tile_pool(name="xraw", bufs=5))
        exp_pool = ctx.enter_context(tc.tile_pool(name="expp", bufs=3))
        smx_pool = ctx.enter_context(tc.tile_pool(name="smx", bufs=3))
        aT_pool = ctx.enter_context(tc.tile_pool(name="aTp", bufs=1))
        at_pool = ctx.enter_context(tc.tile_pool(name="atp", bufs=2))
        os_pool = ctx.enter_context(tc.tile_pool(name="osp", bufs=2))

        # PSUM pools: en 2x2 + m2 2 + aux 2 = 8 banks (proj/fc/pers share m2)
        en_ps = ctx.enter_context(tc.tile_pool(name="en_ps", bufs=2, space="PSUM"))
        m2_ps = ctx.enter_context(tc.tile_pool(name="m2_ps", bufs=2, space="PSUM"))
        aux_ps = ctx.enter_context(tc.tile_pool(name="aux_ps", bufs=2, space="PSUM"))

        # ---- constants ----
        def cload(ap_d, shape, dtype, nm):
            t = const_pool.tile(shape, dtype, tag=nm, name=nm)
            nc.sync.dma_start(t[:], ap_d)
            return t

        wq2t = cload(wq2_d, [128, 128], f16, "wq2t")
        wk2t = cload(wk2_d, [128, 128], f16, "wk2t")
        wv2t = cload(wv2_d, [128, 128], f16, "wv2t")
        pkTt = cload(pkT_d, [HD, NPERS], f16, "pkTt")
        pvt = cload(pv_d, [NPERS, HD], f16, "pvt")
        wpcolt = cload(wpcol_d, [128, HEADS * 4], f32, "wpcolt")
        cgcolt = cload(cgcol_d, [128, 2], f32, "cgcolt")
        w2pt = {nm: cload(d, [128, 256], f32, nm) for nm, d in w2p_d.items()}
        ones1 = const_pool.tile([1, 128], f16, tag="ones1", name="ones1")
        nc.vector.memset(ones1[:], 1.0)

        # ---- persistent activation tiles ----
        qPT = [
            xT_pool.tile([128, SQ], f16, tag=f"qPT{p}", name=f"qPT{p}")
            for p in range(4)
        ]
        kPT = [
            xT_pool.tile([128, KT], f16, tag=f"kPT{p}", name=f"kPT{p}")
            for p in range(4)
        ]
        vS = [
            vs_pool.tile([128, EMBED], f16, tag=f"vS{j}", name=f"vS{j}")
            for j in range(NKJ)
        ]
        vSp = vs_pool.tile([NPERS, EMBED], f16, tag="vSp", name="vSp")
        # q~: per pair, 8 out-head-scaled copies of the group's q'T columns
        qtil = [
            qt_pool.tile([128, HEADS * QG], f16, tag=f"qt{p}", name=f"qt{p}")
            for p in range(4)
        ]

        for p in range(4):
            nc.gpsimd.tensor_copy(kPT[p][0:HD, SEQ:KT], pkTt[:])
            nc.gpsimd.tensor_copy(kPT[p][HD:128, SEQ:KT], pkTt[:])
        for h in range(HEADS):
            nc.gpsimd.tensor_copy(vSp[:, HD * h : HD * (h + 1)], pvt[:])

        # ---- projections (block-diag weights, K=128) ----
        for src_d, nrows, w2, dstT in ((qT_d, SQ, wq2t, qPT), (kT_d, SEQ, wk2t, kPT)):
            for p in range(4):
                rT = xraw_pool.tile([128, SEQ], f16, tag="xr", name="rT")
                nc.sync.dma_start(rT[:, 0:nrows], src_d[128 * p : 128 * (p + 1), :])
                for c in range(nrows // 512):
                    ps = m2_ps.tile([128, 512], f32, tag="m2", name="ps")
                    nc.tensor.matmul(
                        ps[:],
                        lhsT=w2[:],
                        rhs=rT[:, 512 * c : 512 * (c + 1)],
                        start=True,
                        stop=True,
                    )
                    nc.scalar.copy(dstT[p][:, 512 * c : 512 * (c + 1)], ps[:])

        # v: project and transpose back to natural [seq, (h,d)] in one matmul
        # (vT chunk as the stationary operand against block-diag weights).
        for p in range(4):
            rT = xraw_pool.tile([128, SEQ], f16, tag="xr", name="rTv")
            nc.sync.dma_start(rT[:], vT_d[128 * p : 128 * (p + 1), :])
            for j in range(NKJ):
                ps = m2_ps.tile([128, 512], f32, tag="m2", name="psv")
                nc.tensor.matmul(
                    ps[:, 0:128],
                    lhsT=rT[:, 128 * j : 128 * (j + 1)],
                    rhs=wv2t[:],
                    start=True,
                    stop=True,
                )
                nc.scalar.copy(vS[j][:, 128 * p : 128 * (p + 1)], ps[:, 0:128])

        # late-needed constants: issue their DMAs after the projection
        # activations so the rT loads aren't queued behind ~2 MB of tables
        distt = cload(dist_d, [128, DIST_W], f16, "distt")
        fcw = []
        for cc in range(HEADS):
            t = const_pool.tile([HD, EMBED], f16, tag=f"fcw{cc}", name=f"fcw{cc}")
            nc.sync.dma_start(t[:], fcwT_d[HD * cc : HD * (cc + 1), :])
            fcw.append(t)
        fcbt = cload(fcb_d, [1, EMBED], f16, "fcbt")

        # ---- main attention loop ----
        for qg in range(NQG):
            q0g = qg * QG
            # q~ build: W_pre-scaled q'^T copies; columns ordered
            # (qblock, bank, ga, qi) so each matmul's weights are contiguous
            for p in range(4):
                qv = qtil[p].rearrange(
                    "p (b o a q) -> p b o a q", b=QBPG, o=2, a=4, q=QB
                )
                pv_in = qPT[p][:, q0g : q0g + QG].rearrange(
                    "p (b q) -> p b q", b=QBPG, q=QB
                )
                for g in range(HEADS):
                    nc.gpsimd.tensor_scalar_mul(
                        qv[:, :, g // 4, g % 4, :],
                        pv_in[:],
                        wpcolt[:, 4 * g + p : 4 * g + p + 1],
                    )

            aT = aT_pool.tile([128, NKJ * HEADS * QG], f16, tag="aT", name="aT")
            aTv = aT.rearrange("p (j g q) -> p j g q", j=NKJ, g=HEADS, q=QG)
            aTp = aT_pool.tile([NPERS, HEADS * QG], f16, tag="aTp", name="aTp")
            aTpv = aTp.rearrange("p (g q) -> p g q", g=HEADS, q=QG)

            def emit_mix2(qb_i, ex_s, m2b):
                # mix2 transposed: A'^T[k, (g, qi)], two k-subchunks per tile
                for jj in range(NKJ // 2):
                    m2 = m2_ps.tile([128, 512], f32, tag="m2", name="m2")
                    for j2 in range(2):
                        j = 2 * jj + j2
                        for ob in ("A", "B"):
                            nc.tensor.matmul(
                                m2[:, 256 * j2 : 256 * (j2 + 1)],
                                lhsT=ex_s[ob][:, 128 * j : 128 * (j + 1)],
                                rhs=m2b[ob][:],
                                start=(ob == "A"),
                                stop=(ob == "B"),
                            )
                    m2v = m2.rearrange("p (j g q) -> p j g q", j=2, g=HEADS, q=QB)
                    eng = nc.vector if jj % 2 == 0 else nc.scalar
                    ecopy(
                        eng,
                        aTv[:, 2 * jj : 2 * (jj + 1), :, qb_i * QB : (qb_i + 1) * QB],
                        m2v[:],
                    )
                # persistent k rows
                m2 = m2_ps.tile([128, 512], f32, tag="m2", name="m2p")
                for ob in ("A", "B"):
                    nc.tensor.matmul(
                        m2[0:NPERS, 0:256],
                        lhsT=ex_s[ob][:, SEQ:KT],
                        rhs=m2b[ob][:],
                        start=(ob == "A"),
                        stop=(ob == "B"),
                    )
                m2pv = m2[0:NPERS, 0:256].rearrange(
                    "p (g q) -> p g q", g=HEADS, q=QB
                )
                nc.vector.tensor_copy(
                    aTpv[:, :, qb_i * QB : (qb_i + 1) * QB], m2pv[:]
                )

            # software-pipelined over blocks: mix2 of block b is emitted
            # after the scores of block b+1, so the PE streams b+1's scores
            # while the DVE finishes b's denominator chain (reduce ->
            # reciprocal -> m2b build) instead of stalling on it.
            pend = None
            for qb_i in range(QBPG):
                b = qg * QBPG + qb_i
                qc0 = b * QB  # core-local query offset of this block

                # fused scores + W_pre mix + alibi, then exp (1024-wide en
                # tiles: two 512 matmul chunks per tile, one Pool alibi STT
                # and one wide ACT exp per tile)
                # bank A: out-heads 0-3 (rows 32*ga + qi), bank B: 4-7
                ex_s = {}
                acc = smx_pool.tile([128, 6], f32, tag="acc", name="acc")
                for oi, ob in enumerate(("A", "B")):
                    ex_s[ob] = exp_pool.tile(
                        [128, KT], bf16, tag=f"exp{ob}", name=f"ex_s{ob}"
                    )
                    c0 = qb_i * 256 + oi * 128
                    for ci in range(2):
                        en = en_ps.tile([128, 2 * KC], f32, tag="en", name="en")
                        for half in range(2):
                            k0 = (2 * ci + half) * KC
                            for p in range(4):
                                nc.tensor.matmul(
                                    en[:, KC * half : KC * (half + 1)],
                                    lhsT=qtil[p][:, c0 : c0 + 128],
                                    rhs=kPT[p][:, k0 : k0 + KC],
                                    start=(p == 0),
                                    stop=(p == 3),
                                )
                        # en += c_g * dist, on idle Pool engine
                        u0 = SEQ + ci * 2 * KC - qc0
                        nc.vector.scalar_tensor_tensor(
                            en[:],
                            distt[:, u0 : u0 + 2 * KC],
                            cgcolt[:, oi : oi + 1],
                            en[:],
                            op0=mybir.AluOpType.mult,
                            op1=mybir.AluOpType.add,
                        )
                        nc.scalar.activation(
                            ex_s[ob][:, 2 * KC * ci : 2 * KC * (ci + 1)],
                            en[:],
                            EXP,
                            scale=SCALE,
                            accum_out=acc[:, 3 * oi + ci : 3 * oi + ci + 1],
                        )
                    # persistent k columns (no alibi)
                    enp = m2_ps.tile([128, 512], f32, tag="m2", name="enp")
                    for p in range(4):
                        nc.tensor.matmul(
                            enp[:, 0:NPERS],
                            lhsT=qtil[p][:, c0 : c0 + 128],
                            rhs=kPT[p][:, SEQ:KT],
                            start=(p == 0),
                            stop=(p == 3),
                        )
                    nc.scalar.activation(
                        ex_s[ob][:, SEQ:KT],
                        enp[:, 0:NPERS],
                        EXP,
                        scale=SCALE,
                        accum_out=acc[:, 3 * oi + 2 : 3 * oi + 3],
                    )

                # softmax denominators; fold 1/sum into the mix2 matrices
                m2b = {}
                for oi, ob in enumerate(("A", "B")):
                    sm = smx_pool.tile([128, 2], f32, tag=f"sm{ob}", name=f"sm{ob}")
                    nc.vector.reduce_sum(
                        sm[:, 0:1],
                        acc[:, 3 * oi : 3 * oi + 3],
                        axis=X,
                    )
                    rc = smx_pool.tile([128, 1], f32, tag=f"rc{ob}", name=f"rc{ob}")
                    nc.vector.reciprocal(rc[:], sm[:, 0:1])
                    m2b[ob] = smx_pool.tile(
                        [128, 256], bf16, tag=f"m2b{ob}", name=f"m2b{ob}"
                    )
                    nc.gpsimd.tensor_scalar_mul(
                        m2b[ob][:], w2pt[f"w2p{ob}"][:], rc[:, 0:1]
                    )

                if pend is not None:
                    emit_mix2(*pend)
                pend = (qb_i, ex_s, m2b)
            emit_mix2(*pend)

            # ---- A@V for the group: attn^T[d, q] per out-head g ----
            # two av accumulation regions per PSUM bank: with bufs=2 the
            # copy-out of head g's result only gates head g+4's matmuls
            # (not g+2's), hiding the aT/at_s copy backlog on DVE/ACT
            at_s = []
            auxt = None
            for g in range(HEADS):
                if g % 2 == 0:
                    auxt = aux_ps.tile([128, 2 * QG], f32, tag="aux", name="av2")
                av = auxt[:, QG * (g % 2) : QG * (g % 2 + 1)]
                for j in range(NKJ):
                    nc.tensor.matmul(
                        av[0:HD, :],
                        lhsT=vS[j][:, HD * g : HD * (g + 1)],
                        rhs=aTv[:, j, g, :],
                        start=(j == 0),
                        stop=False,
                    )
                nc.tensor.matmul(
                    av[0:HD, :],
                    lhsT=vSp[:, HD * g : HD * (g + 1)],
                    rhs=aTpv[:, g, :],
                    start=False,
                    stop=True,
                )
                ats = at_pool.tile([HD, QG], f16, tag=f"ats{g}", name=f"ats{g}")
                eng = nc.scalar if g % 2 == 0 else nc.vector
                ecopy(eng, ats[:], av[0:HD, :])
                at_s.append(ats)

            # ---- fc_out: 8 accumulated K=64 matmuls + rank-1 bias ----
            # both 128-row output halves accumulate interleaved per head, so
            # the first fc matmul only waits on head 0's at_s copy (not 7's)
            fps = [
                m2_ps.tile([128, EMBED], f32, tag="m2", name=f"fp{sub}")
                for sub in (0, 1)
            ]
            for g in range(HEADS):
                for sub in (0, 1):
                    nc.tensor.matmul(
                        fps[sub][:],
                        lhsT=at_s[g][:, 128 * sub : 128 * (sub + 1)],
                        rhs=fcw[g][:],
                        start=(g == 0),
                        stop=False,
                    )
            for sub in (0, 1):
                nc.tensor.matmul(
                    fps[sub][:], lhsT=ones1[:], rhs=fcbt[:], start=False,
                    stop=True,
                )
                o_s = os_pool.tile([128, EMBED], f32, tag="os", name="o_s")
                nc.vector.tensor_copy(o_s[:], fps[sub][:])
                q_row = qg * QG + sub * 128
                nc.sync.dma_start(out_d[q_row : q_row + 128, :], o_s[:])

    nc.compile()
    return nc


def _get_nc():
    if "nc" not in _CACHED:
        _CACHED["nc"] = build_bass()
    return _CACHED["nc"]


def kernel(
    values,
    keys,
    queries,
    mask,
    Wv,
    Wk,
    Wq,
    W_pre,
    W_post,
    p_keys,
    p_values,
    fc_w,
    fc_b,
):
    """Full-input entry point. mask is all-True per the problem spec
    (fill: ones) and is therefore not consumed on-device."""
    from concourse.bass_utils import run_bass_kernel_spmd

    qT = np.asarray(queries, np.float32).astype(np.float16).transpose(0, 2, 1)
    kT = np.asarray(keys, np.float32).astype(np.float16).transpose(0, 2, 1)
    vT = np.asarray(values, np.float32).astype(np.float16).transpose(0, 2, 1)
    consts = _host_consts(
        np.asarray(Wv, np.float32),
        np.asarray(Wk, np.float32),
        np.asarray(Wq, np.float32),
        np.asarray(W_pre, np.float32),
        np.asarray(W_post, np.float32),
        np.asarray(p_keys, np.float32),
        np.asarray(p_values, np.float32),
        np.asarray(fc_w, np.float32),
        np.asarray(fc_b, np.float32),
    )

    nc = _get_nc()
    in_maps = []
    for core in range(N_CORES):
        n, half = core // 2, core % 2
        qbase = half * SQ
        m = {
            "qT16": np.ascontiguousarray(qT[n, :, qbase : qbase + SQ]),
            "kT16": np.ascontiguousarray(kT[n]),
            "vT16": np.ascontiguousarray(vT[n]),
            "dist": _dist_table(qbase),
        }
        m.update(consts)
        in_maps.append(m)

    global _last_in_maps
    _last_in_maps = in_maps
    res = run_bass_kernel_spmd(nc, in_maps, core_ids=list(range(N_CORES)))
    out = np.empty((N_BATCH, SEQ, EMBED), np.float32)
    for core in range(N_CORES):
        n, half = core // 2, core % 2
        out[n, half * SQ : (half + 1) * SQ, :] = res.results[core]["out"]
    return out



# revision 16
# speedup vs baseline: 1.8518x; 1.8518x over previous
"""TRN2 Bass/Tile kernel for NeoTCNAttention (talking-heads attention with
ALiBi + persistent memory), SPMD over 8 NeuronCores.

Sharding: data-parallel over batch N=4 x 2 halves of the query axis
(each core: one batch element, 1024 query positions, full keys/values).
No collectives: every core computes a disjoint slab of the output.

Per-core pipeline (all matmuls are standard full-array ops):
  - activations arrive transposed/fp16 (x^T pair tiles [128=(2 heads x 64
    dims), seq]); Q/K/V projections use host-built block-diag(W^T, W^T)
    weights, one matmul per 512-wide chunk.
  - W_pre (pre-softmax talking heads) is folded into the score matmuls:
    q~[(h,d), (g, qi)] = W_pre[g, h] * q'^T, so for each output bank
    (4 g-heads x 32 q rows) the scores+mix are 4 accumulated pair matmuls
    plus one ALiBi matmul against a distance table (ALiBi slopes folded
    through W_pre into per-g scalars c_g).
  - exp runs on the scalar engine straight out of PSUM with accumulated
    row sums; softmax normalization is folded into the post-softmax
    talking-heads matrix (per-row 1/sum scaling).
  - post-softmax talking heads run as transposing matmuls (exp tile as
    the stationary operand) producing A'^T [k, (g, q)] directly, which
    feeds A@V with v' columns as stationary weights.
  - fc_out accumulates 8 K=64 matmuls with the bias folded in as a
    rank-1 (ones x fc_b) matmul.
"""

import math

import numpy as np

# ---- problem constants (hardcoded per spec) ----
N_BATCH = 4
SEQ = 2048
EMBED = 512
HEADS = 8
HD = 64
NPERS = 16
KT = SEQ + NPERS  # 2064
ALIBI_ALPHA = 1.25
START_I = 1
N_CORES = 8
SQ = 1024  # query positions per core
SCALE = 1.0 / math.sqrt(EMBED)

QB = 32            # query positions per block (bank rows = 4 g x 32 qi)
NQB = SQ // QB     # 32 blocks
QG = 256           # query positions per A@V group
NQG = SQ // QG     # 4 groups
QBPG = QG // QB    # 8 blocks per group
KC = 512           # k chunk width for scores/exp
NKC = SEQ // KC    # 4 (persistent 16 handled separately)
NKJ = SEQ // 128   # 16 mix2/A@V k-subchunks (+1 partial of NPERS)
DIST_W = SEQ + KT  # 4112 distance-table width

_CACHED = {}
_last_in_maps = None


def _host_consts(Wv, Wk, Wq, W_pre, W_post, p_keys, p_values, fc_w, fc_b):
    """Derived constant tensors shipped to the device (layout prep only)."""
    f16 = np.float16
    Z = np.zeros((HD, HD), np.float32)
    c = {}
    c["wq2"] = np.block([[Wq.T, Z], [Z, Wq.T]]).astype(f16)  # [128, 128]
    c["wk2"] = np.block([[Wk.T, Z], [Z, Wk.T]]).astype(f16)
    c["wv2"] = np.block([[Wv.T, Z], [Z, Wv.T]]).astype(f16)
    c["pkT"] = np.ascontiguousarray(p_keys[:, 0, :].T).astype(f16)   # [64, 16]
    c["pv"] = np.ascontiguousarray(p_values[:, 0, :]).astype(f16)    # [16, 64]

    # W_pre scaling columns for the q~ build: col 4*g + p scales pair p's
    # rows (head 2p in partitions 0-63, head 2p+1 in 64-127) for out-head g.
    wpcol = np.zeros((128, HEADS * 4), np.float32)
    for g in range(HEADS):
        for p in range(4):
            wpcol[0:HD, 4 * g + p] = W_pre[g, 2 * p]
            wpcol[HD:128, 4 * g + p] = W_pre[g, 2 * p + 1]
    c["wpcol"] = wpcol

    slopes = 2.0 ** (
        -ALIBI_ALPHA * (np.arange(1, HEADS + 1, dtype=np.float64) + START_I)
    )
    cg_vec = -(W_pre.astype(np.float64) @ slopes)  # per-out-head slope mix

    cgcol = np.zeros((128, 2), np.float32)
    for oi in range(2):
        for ga in range(4):
            cgcol[32 * ga : 32 * (ga + 1), oi] = cg_vec[4 * oi + ga]
    c["cgcol"] = cgcol

    idx = np.arange(QB)
    for ob, gofs in (("A", 0), ("B", 4)):
        w2p = np.zeros((128, 256), np.float32)
        for ga in range(4):
            for g2 in range(HEADS):
                w2p[32 * ga + idx, 32 * g2 + idx] = W_post[g2, gofs + ga]
        c[f"w2p{ob}"] = w2p.astype(np.float32)

    c["fcwT"] = np.ascontiguousarray(fc_w.T).astype(f16)  # [512, 512] rows=(g,d)
    c["fcb"] = fc_b.reshape(1, EMBED).astype(f16)
    return c


def _dist_table(qbase: int) -> np.ndarray:
    """T[32*ga + qi, u] = |qbase + qi - (u - SEQ)| as fp16 (rows replicated
    over the 4 out-heads of a bank; exact for ints <= 2048)."""
    u = np.arange(DIST_W)
    qi = np.arange(QB)
    t = np.abs(qbase + qi[:, None] - (u[None, :] - SEQ)).astype(np.float16)
    return np.tile(t, (4, 1))


def build_bass():
    import concourse.mybir as mybir
    import concourse.tile as tile
    from concourse import bacc
    from contextlib import ExitStack

    f32 = mybir.dt.float32
    f16 = mybir.dt.float16
    bf16 = mybir.dt.bfloat16
    EXP = mybir.ActivationFunctionType.Exp
    X = mybir.AxisListType.X

    nc = bacc.Bacc(
        "TRN2", target_bir_lowering=False, debug=False, num_devices=N_CORES
    )

    # ---- DRAM I/O ----
    qT_d = nc.dram_tensor("qT16", [EMBED, SQ], f16, kind="ExternalInput").ap()
    kT_d = nc.dram_tensor("kT16", [EMBED, SEQ], f16, kind="ExternalInput").ap()
    vT_d = nc.dram_tensor("vT16", [EMBED, SEQ], f16, kind="ExternalInput").ap()
    wq2_d = nc.dram_tensor("wq2", [128, 128], f16, kind="ExternalInput").ap()
    wk2_d = nc.dram_tensor("wk2", [128, 128], f16, kind="ExternalInput").ap()
    wv2_d = nc.dram_tensor("wv2", [128, 128], f16, kind="ExternalInput").ap()
    pkT_d = nc.dram_tensor("pkT", [HD, NPERS], f16, kind="ExternalInput").ap()
    pv_d = nc.dram_tensor("pv", [NPERS, HD], f16, kind="ExternalInput").ap()
    wpcol_d = nc.dram_tensor(
        "wpcol", [128, HEADS * 4], f32, kind="ExternalInput"
    ).ap()
    cgcol_d = nc.dram_tensor("cgcol", [128, 2], f32, kind="ExternalInput").ap()
    w2p_d = {
        nm: nc.dram_tensor(nm, [128, 256], f32, kind="ExternalInput").ap()
        for nm in ("w2pA", "w2pB")
    }
    dist_d = nc.dram_tensor("dist", [128, DIST_W], f16, kind="ExternalInput").ap()
    fcwT_d = nc.dram_tensor("fcwT", [EMBED, EMBED], f16, kind="ExternalInput").ap()
    fcb_d = nc.dram_tensor("fcb", [1, EMBED], f16, kind="ExternalInput").ap()
    out_d = nc.dram_tensor("out", [SQ, EMBED], f32, kind="ExternalOutput").ap()

    def ecopy(eng, out, in_):
        if eng is nc.scalar:
            eng.copy(out, in_)
        else:
            eng.tensor_copy(out, in_)

    with tile.TileContext(nc) as tc, ExitStack() as ctx:
        const_pool = ctx.enter_context(tc.tile_pool(name="const", bufs=1))
        xT_pool = ctx.enter_context(tc.tile_pool(name="xT", bufs=1))
        vs_pool = ctx.enter_context(tc.tile_pool(name="vs", bufs=1))
        qt_pool = ctx.enter_context(tc.tile_pool(name="qt", bufs=1))
        xraw_pool = ctx.enter_context(tc.tile_pool(name="xraw", bufs=5))
        exp_pool = ctx.enter_context(tc.tile_pool(name="expp", bufs=3))
        smx_pool = ctx.enter_context(tc.tile_pool(name="smx", bufs=3))
        aT_pool = ctx.enter_context(tc.tile_pool(name="aTp", bufs=1))
        at_pool = ctx.enter_context(tc.tile_pool(name="atp", bufs=2))
        os_pool = ctx.enter_context(tc.tile_pool(name="osp", bufs=2))

        # PSUM pools: en 2x2 + m2 2 + aux 2 = 8 banks (proj/fc/pers share m2)
        en_ps = ctx.enter_context(tc.tile_pool(name="en_ps", bufs=2, space="PSUM"))
        m2_ps = ctx.enter_context(tc.tile_pool(name="m2_ps", bufs=2, space="PSUM"))
        aux_ps = ctx.enter_context(tc.tile_pool(name="aux_ps", bufs=2, space="PSUM"))

        # ---- constants ----
        def cload(ap_d, shape, dtype, nm):
            t = const_pool.tile(shape, dtype, tag=nm, name=nm)
            nc.sync.dma_start(t[:], ap_d)
            return t

        wq2t = cload(wq2_d, [128, 128], f16, "wq2t")
        wk2t = cload(wk2_d, [128, 128], f16, "wk2t")
        wv2t = cload(wv2_d, [128, 128], f16, "wv2t")
        pkTt = cload(pkT_d, [HD, NPERS], f16, "pkTt")
        pvt = cload(pv_d, [NPERS, HD], f16, "pvt")
        wpcolt = cload(wpcol_d, [128, HEADS * 4], f32, "wpcolt")
        cgcolt = cload(cgcol_d, [128, 2], f32, "cgcolt")
        w2pt = {nm: cload(d, [128, 256], f32, nm) for nm, d in w2p_d.items()}
        ones1 = const_pool.tile([1, 128], f16, tag="ones1", name="ones1")
        nc.vector.memset(ones1[:], 1.0)

        # ---- persistent activation tiles ----
        qPT = [
            xT_pool.tile([128, SQ], f16, tag=f"qPT{p}", name=f"qPT{p}")
            for p in range(4)
        ]
        kPT = [
            xT_pool.tile([128, KT], f16, tag=f"kPT{p}", name=f"kPT{p}")
            for p in range(4)
        ]
        vS = [
            vs_pool.tile([128, EMBED], f16, tag=f"vS{j}", name=f"vS{j}")
            for j in range(NKJ)
        ]
        vSp = vs_pool.tile([NPERS, EMBED], f16, tag="vSp", name="vSp")
        # q~: per pair, 8 out-head-scaled copies of the group's q'T columns
        qtil = [
            qt_pool.tile([128, HEADS * QG], f16, tag=f"qt{p}", name=f"qt{p}")
            for p in range(4)
        ]

        for p in range(4):
            nc.vector.tensor_copy(kPT[p][0:HD, SEQ:KT], pkTt[:])
            nc.vector.tensor_copy(kPT[p][HD:128, SEQ:KT], pkTt[:])
        for h in range(HEADS):
            nc.vector.tensor_copy(vSp[:, HD * h : HD * (h + 1)], pvt[:])

        # ---- projections (block-diag weights, K=128) ----
        for src_d, nrows, w2, dstT in ((qT_d, SQ, wq2t, qPT), (kT_d, SEQ, wk2t, kPT)):
            for p in range(4):
                rT = xraw_pool.tile([128, SEQ], f16, tag="xr", name="rT")
                nc.sync.dma_start(rT[:, 0:nrows], src_d[128 * p : 128 * (p + 1), :])
                for c in range(nrows // 512):
                    ps = m2_ps.tile([128, 512], f32, tag="m2", name="ps")
                    nc.tensor.matmul(
                        ps[:],
                        lhsT=w2[:],
                        rhs=rT[:, 512 * c : 512 * (c + 1)],
                        start=True,
                        stop=True,
                    )
                    nc.scalar.copy(dstT[p][:, 512 * c : 512 * (c + 1)], ps[:])

        # v: project and transpose back to natural [seq, (h,d)] in one matmul
        # (vT chunk as the stationary operand against block-diag weights).
        for p in range(4):
            rT = xraw_pool.tile([128, SEQ], f16, tag="xr", name="rTv")
            nc.sync.dma_start(rT[:], vT_d[128 * p : 128 * (p + 1), :])
            for j in range(NKJ):
                ps = m2_ps.tile([128, 512], f32, tag="m2", name="psv")
                nc.tensor.matmul(
                    ps[:, 0:128],
                    lhsT=rT[:, 128 * j : 128 * (j + 1)],
                    rhs=wv2t[:],
                    start=True,
                    stop=True,
                )
                nc.scalar.copy(vS[j][:, 128 * p : 128 * (p + 1)], ps[:, 0:128])

        # late-needed constants: issue their DMAs after the projection
        # activations so the rT loads aren't queued behind ~2 MB of tables
        distt = cload(dist_d, [128, DIST_W], f16, "distt")
        fcw = []
        for cc in range(HEADS):
            t = const_pool.tile([HD, EMBED], f16, tag=f"fcw{cc}", name=f"fcw{cc}")
            nc.sync.dma_start(t[:], fcwT_d[HD * cc : HD * (cc + 1), :])
            fcw.append(t)
        fcbt = cload(fcb_d, [1, EMBED], f16, "fcbt")

        # ---- main attention loop ----
        for qg in range(NQG):
            q0g = qg * QG
            # q~ build: W_pre-scaled q'^T copies; columns ordered
            # (qblock, bank, ga, qi) so each matmul's weights are contiguous
            for p in range(4):
                qv = qtil[p].rearrange(
                    "p (b o a q) -> p b o a q", b=QBPG, o=2, a=4, q=QB
                )
                pv_in = qPT[p][:, q0g : q0g + QG].rearrange(
                    "p (b q) -> p b q", b=QBPG, q=QB
                )
                for g in range(HEADS):
                    nc.vector.tensor_scalar_mul(
                        qv[:, :, g // 4, g % 4, :],
                        pv_in[:],
                        wpcolt[:, 4 * g + p : 4 * g + p + 1],
                    )

            aT = aT_pool.tile([128, NKJ * HEADS * QG], f16, tag="aT", name="aT")
            aTv = aT.rearrange("p (j g q) -> p j g q", j=NKJ, g=HEADS, q=QG)
            aTp = aT_pool.tile([NPERS, HEADS * QG], f16, tag="aTp", name="aTp")
            aTpv = aTp.rearrange("p (g q) -> p g q", g=HEADS, q=QG)

            def emit_mix2(qb_i, ex_s, m2b):
                # mix2 transposed: A'^T[k, (g, qi)], two k-subchunks per tile
                for jj in range(NKJ // 2):
                    m2 = m2_ps.tile([128, 512], f32, tag="m2", name="m2")
                    for j2 in range(2):
                        j = 2 * jj + j2
                        for ob in ("A", "B"):
                            nc.tensor.matmul(
                                m2[:, 256 * j2 : 256 * (j2 + 1)],
                                lhsT=ex_s[ob][:, 128 * j : 128 * (j + 1)],
                                rhs=m2b[ob][:],
                                start=(ob == "A"),
                                stop=(ob == "B"),
                            )
                    m2v = m2.rearrange("p (j g q) -> p j g q", j=2, g=HEADS, q=QB)
                    eng = nc.vector if jj % 2 == 0 else nc.scalar
                    ecopy(
                        eng,
                        aTv[:, 2 * jj : 2 * (jj + 1), :, qb_i * QB : (qb_i + 1) * QB],
                        m2v[:],
                    )
                # persistent k rows
                m2 = m2_ps.tile([128, 512], f32, tag="m2", name="m2p")
                for ob in ("A", "B"):
                    nc.tensor.matmul(
                        m2[0:NPERS, 0:256],
                        lhsT=ex_s[ob][:, SEQ:KT],
                        rhs=m2b[ob][:],
                        start=(ob == "A"),
                        stop=(ob == "B"),
                    )
                m2pv = m2[0:NPERS, 0:256].rearrange(
                    "p (g q) -> p g q", g=HEADS, q=QB
                )
                nc.vector.tensor_copy(
                    aTpv[:, :, qb_i * QB : (qb_i + 1) * QB], m2pv[:]
                )

            # software-pipelined over blocks: mix2 of block b is emitted
            # after the scores of block b+1, so the PE streams b+1's scores
            # while the DVE finishes b's denominator chain (reduce ->
            # reciprocal -> m2b build) instead of stalling on it.
            pend = None
            for qb_i in range(QBPG):
                b = qg * QBPG + qb_i
                qc0 = b * QB  # core-local query offset of this block

                # fused scores + W_pre mix + alibi, then exp (1024-wide en
                # tiles: two 512 matmul chunks per tile, one Pool alibi STT
                # and one wide ACT exp per tile)
                # bank A: out-heads 0-3 (rows 32*ga + qi), bank B: 4-7
                ex_s = {}
                acc = smx_pool.tile([128, 6], f32, tag="acc", name="acc")
                for oi, ob in enumerate(("A", "B")):
                    ex_s[ob] = exp_pool.tile(
                        [128, KT], bf16, tag=f"exp{ob}", name=f"ex_s{ob}"
                    )
                    c0 = qb_i * 256 + oi * 128
                    for ci in range(2):
                        en = en_ps.tile([128, 2 * KC], f32, tag="en", name="en")
                        for half in range(2):
                            k0 = (2 * ci + half) * KC
                            for p in range(4):
                                nc.tensor.matmul(
                                    en[:, KC * half : KC * (half + 1)],
                                    lhsT=qtil[p][:, c0 : c0 + 128],
                                    rhs=kPT[p][:, k0 : k0 + KC],
                                    start=(p == 0),
                                    stop=(p == 3),
                                )
                        # en += c_g * dist, on idle Pool engine
                        u0 = SEQ + ci * 2 * KC - qc0
                        nc.vector.scalar_tensor_tensor(
                            en[:],
                            distt[:, u0 : u0 + 2 * KC],
                            cgcolt[:, oi : oi + 1],
                            en[:],
                            op0=mybir.AluOpType.mult,
                            op1=mybir.AluOpType.add,
                        )
                        nc.scalar.activation(
                            ex_s[ob][:, 2 * KC * ci : 2 * KC * (ci + 1)],
                            en[:],
                            EXP,
                            scale=SCALE,
                            accum_out=acc[:, 3 * oi + ci : 3 * oi + ci + 1],
                        )
                    # persistent k columns (no alibi)
                    enp = m2_ps.tile([128, 512], f32, tag="m2", name="enp")
                    for p in range(4):
                        nc.tensor.matmul(
                            enp[:, 0:NPERS],
                            lhsT=qtil[p][:, c0 : c0 + 128],
                            rhs=kPT[p][:, SEQ:KT],
                            start=(p == 0),
                            stop=(p == 3),
                        )
                    nc.scalar.activation(
                        ex_s[ob][:, SEQ:KT],
                        enp[:, 0:NPERS],
                        EXP,
                        scale=SCALE,
                        accum_out=acc[:, 3 * oi + 2 : 3 * oi + 3],
                    )

                # softmax denominators; fold 1/sum into the mix2 matrices
                m2b = {}
                for oi, ob in enumerate(("A", "B")):
                    sm = smx_pool.tile([128, 2], f32, tag=f"sm{ob}", name=f"sm{ob}")
                    nc.vector.reduce_sum(
                        sm[:, 0:1],
                        acc[:, 3 * oi : 3 * oi + 3],
                        axis=X,
                    )
                    rc = smx_pool.tile([128, 1], f32, tag=f"rc{ob}", name=f"rc{ob}")
                    nc.vector.reciprocal(rc[:], sm[:, 0:1])
                    m2b[ob] = smx_pool.tile(
                        [128, 256], bf16, tag=f"m2b{ob}", name=f"m2b{ob}"
                    )
                    nc.vector.tensor_scalar_mul(
                        m2b[ob][:], w2pt[f"w2p{ob}"][:], rc[:, 0:1]
                    )

                if pend is not None:
                    emit_mix2(*pend)
                pend = (qb_i, ex_s, m2b)
            emit_mix2(*pend)

            # ---- A@V for the group: attn^T[d, q] per out-head g ----
            # two av accumulation regions per PSUM bank: with bufs=2 the
            # copy-out of head g's result only gates head g+4's matmuls
            # (not g+2's), hiding the aT/at_s copy backlog on DVE/ACT
            at_s = []
            auxt = None
            for g in range(HEADS):
                if g % 2 == 0:
                    auxt = aux_ps.tile([128, 2 * QG], f32, tag="aux", name="av2")
                av = auxt[:, QG * (g % 2) : QG * (g % 2 + 1)]
                for j in range(NKJ):
                    nc.tensor.matmul(
                        av[0:HD, :],
                        lhsT=vS[j][:, HD * g : HD * (g + 1)],
                        rhs=aTv[:, j, g, :],
                        start=(j == 0),
                        stop=False,
                    )
                nc.tensor.matmul(
                    av[0:HD, :],
                    lhsT=vSp[:, HD * g : HD * (g + 1)],
                    rhs=aTpv[:, g, :],
                    start=False,
                    stop=True,
                )
                ats = at_pool.tile([HD, QG], f16, tag=f"ats{g}", name=f"ats{g}")
                eng = nc.scalar if g % 2 == 0 else nc.vector
                ecopy(eng, ats[:], av[0:HD, :])
                at_s.append(ats)

            # ---- fc_out: 8 accumulated K=64 matmuls + rank-1 bias ----
            # both 128-row output halves accumulate interleaved per head, so
            # the first fc matmul only waits on head 0's at_s copy (not 7's)
            fps = [
                m2_ps.tile([128, EMBED], f32, tag="m2", name=f"fp{sub}")
                for sub in (0, 1)
            ]
            for g in range(HEADS):
                for sub in (0, 1):
                    nc.tensor.matmul(
                        fps[sub][:],
                        lhsT=at_s[g][:, 128 * sub : 128 * (sub + 1)],
                        rhs=fcw[g][:],
                        start=(g == 0),
                        stop=False,
                    )
            for sub in (0, 1):
                nc.tensor.matmul(
                    fps[sub][:], lhsT=ones1[:], rhs=fcbt[:], start=False,
                    stop=True,
                )
                o_s = os_pool.tile([128, EMBED], f32, tag="os", name="o_s")
                nc.vector.tensor_copy(o_s[:], fps[sub][:])
                q_row = qg * QG + sub * 128
                nc.sync.dma_start(out_d[q_row : q_row + 128, :], o_s[:])

    nc.compile()
    return nc


def _get_nc():
    if "nc" not in _CACHED:
        _CACHED["nc"] = build_bass()
    return _CACHED["nc"]


def kernel(
    values,
    keys,
    queries,
    mask,
    Wv,
    Wk,
    Wq,
    W_pre,
    W_post,
    p_keys,
    p_values,
    fc_w,
    fc_b,
):
    """Full-input entry point. mask is all-True per the problem spec
    (fill: ones) and is therefore not consumed on-device."""
    from concourse.bass_utils import run_bass_kernel_spmd

    qT = np.asarray(queries, np.float32).astype(np.float16).transpose(0, 2, 1)
    kT = np.asarray(keys, np.float32).astype(np.float16).transpose(0, 2, 1)
    vT = np.asarray(values, np.float32).astype(np.float16).transpose(0, 2, 1)
    consts = _host_consts(
        np.asarray(Wv, np.float32),
        np.asarray(Wk, np.float32),
        np.asarray(Wq, np.float32),
        np.asarray(W_pre, np.float32),
        np.asarray(W_post, np.float32),
        np.asarray(p_keys, np.float32),
        np.asarray(p_values, np.float32),
        np.asarray(fc_w, np.float32),
        np.asarray(fc_b, np.float32),
    )

    nc = _get_nc()
    in_maps = []
    for core in range(N_CORES):
        n, half = core // 2, core % 2
        qbase = half * SQ
        m = {
            "qT16": np.ascontiguousarray(qT[n, :, qbase : qbase + SQ]),
            "kT16": np.ascontiguousarray(kT[n]),
            "vT16": np.ascontiguousarray(vT[n]),
            "dist": _dist_table(qbase),
        }
        m.update(consts)
        in_maps.append(m)

    global _last_in_maps
    _last_in_maps = in_maps
    res = run_bass_kernel_spmd(nc, in_maps, core_ids=list(range(N_CORES)))
    out = np.empty((N_BATCH, SEQ, EMBED), np.float32)
    for core in range(N_CORES):
        n, half = core // 2, core % 2
        out[n, half * SQ : (half + 1) * SQ, :] = res.results[core]["out"]
    return out



# revision 19
# speedup vs baseline: 1.8519x; 1.0001x over previous
"""TRN2 Bass/Tile kernel for NeoTCNAttention (talking-heads attention with
ALiBi + persistent memory), SPMD over 8 NeuronCores.

Sharding: data-parallel over batch N=4 x 2 halves of the query axis
(each core: one batch element, 1024 query positions, full keys/values).
No collectives: every core computes a disjoint slab of the output.

Per-core pipeline (all matmuls are standard full-array ops):
  - activations arrive transposed/fp16 (x^T pair tiles [128=(2 heads x 64
    dims), seq]); Q/K/V projections use host-built block-diag(W^T, W^T)
    weights, one matmul per 512-wide chunk.  The k projection is woven
    into block 0's score-chunk loop and the v projection is spread over
    blocks 1-5 so the PE never idles in a projection prologue.
  - W_pre (pre-softmax talking heads) is folded into the score matmuls:
    q~[(h,d), (g, qi)] = W_pre[g, h] * q'^T, so for each output bank
    (4 g-heads x 32 q rows) the scores+mix are 4 accumulated pair matmuls;
    ALiBi lands as a DVE scalar_tensor_tensor against a distance table
    (slopes folded through W_pre into per-g scalars c_g).
  - exp runs on the scalar engine straight out of PSUM with accumulated
    row sums; softmax normalization is folded into the post-softmax
    talking-heads matrix (per-row 1/sum scaling).
  - q~ tiles are double-buffered: the next group's W_pre-scaled copies are
    built on the DVE during blocks 2-5 of the current group.
  - post-softmax talking heads run as transposing matmuls (exp tile as
    the stationary operand) producing A'^T [k, (g, q)] directly, which
    feeds A@V with v' columns as stationary weights.
  - fc_out accumulates 8 K=64 matmuls with the bias folded in as a
    rank-1 (ones x fc_b) matmul.
"""

import math

import numpy as np

# ---- problem constants (hardcoded per spec) ----
N_BATCH = 4
SEQ = 2048
EMBED = 512
HEADS = 8
HD = 64
NPERS = 16
KT = SEQ + NPERS  # 2064
ALIBI_ALPHA = 1.25
START_I = 1
N_CORES = 8
SQ = 1024  # query positions per core
SCALE = 1.0 / math.sqrt(EMBED)

QB = 32            # query positions per block (bank rows = 4 g x 32 qi)
NQB = SQ // QB     # 32 blocks
QG = 256           # query positions per A@V group
NQG = SQ // QG     # 4 groups
QBPG = QG // QB    # 8 blocks per group
KC = 512           # k chunk width for scores/exp
NKC = SEQ // KC    # 4 (persistent 16 handled separately)
NKJ = SEQ // 128   # 16 mix2/A@V k-subchunks (+1 partial of NPERS)
TOFS = 1024        # distance-table column offset (u >= 1056 always)
DIST_W = SEQ + KT - TOFS  # 3088 stored distance-table width

_CACHED = {}
_last_in_maps = None


def _host_consts(Wv, Wk, Wq, W_pre, W_post, p_keys, p_values, fc_w, fc_b):
    """Derived constant tensors shipped to the device (layout prep only)."""
    f16 = np.float16
    Z = np.zeros((HD, HD), np.float32)
    c = {}
    c["wq2"] = np.block([[Wq.T, Z], [Z, Wq.T]]).astype(f16)  # [128, 128]
    c["wk2"] = np.block([[Wk.T, Z], [Z, Wk.T]]).astype(f16)
    c["wv2"] = np.block([[Wv.T, Z], [Z, Wv.T]]).astype(f16)
    c["pkT"] = np.ascontiguousarray(p_keys[:, 0, :].T).astype(f16)   # [64, 16]
    c["pv"] = np.ascontiguousarray(p_values[:, 0, :]).astype(f16)    # [16, 64]

    # W_pre scaling columns for the q~ build: col 4*g + p scales pair p's
    # rows (head 2p in partitions 0-63, head 2p+1 in 64-127) for out-head g.
    wpcol = np.zeros((128, HEADS * 4), np.float32)
    for g in range(HEADS):
        for p in range(4):
            wpcol[0:HD, 4 * g + p] = W_pre[g, 2 * p]
            wpcol[HD:128, 4 * g + p] = W_pre[g, 2 * p + 1]
    c["wpcol"] = wpcol

    slopes = 2.0 ** (
        -ALIBI_ALPHA * (np.arange(1, HEADS + 1, dtype=np.float64) + START_I)
    )
    cg_vec = -(W_pre.astype(np.float64) @ slopes)  # per-out-head slope mix

    cgcol = np.zeros((128, 2), np.float32)
    for oi in range(2):
        for ga in range(4):
            cgcol[32 * ga : 32 * (ga + 1), oi] = cg_vec[4 * oi + ga]
    c["cgcol"] = cgcol

    idx = np.arange(QB)
    for ob, gofs in (("A", 0), ("B", 4)):
        w2p = np.zeros((128, 256), np.float32)
        for ga in range(4):
            for g2 in range(HEADS):
                w2p[32 * ga + idx, 32 * g2 + idx] = W_post[g2, gofs + ga]
        c[f"w2p{ob}"] = w2p.astype(np.float32)

    c["fcwT"] = np.ascontiguousarray(fc_w.T).astype(f16)  # [512, 512] rows=(g,d)
    c["fcb"] = fc_b.reshape(1, EMBED).astype(f16)
    return c


def _dist_table(qbase: int) -> np.ndarray:
    """T[32*ga + qi, u - TOFS] = |qbase + qi - (u - SEQ)| as fp16 (rows
    replicated over the 4 out-heads of a bank; exact for ints <= 2048)."""
    u = np.arange(TOFS, TOFS + DIST_W)
    qi = np.arange(QB)
    t = np.abs(qbase + qi[:, None] - (u[None, :] - SEQ)).astype(np.float16)
    return np.tile(t, (4, 1))


def build_bass():
    import concourse.mybir as mybir
    import concourse.tile as tile
    from concourse import bacc
    from contextlib import ExitStack

    f32 = mybir.dt.float32
    f16 = mybir.dt.float16
    bf16 = mybir.dt.bfloat16
    EXP = mybir.ActivationFunctionType.Exp
    X = mybir.AxisListType.X

    nc = bacc.Bacc(
        "TRN2", target_bir_lowering=False, debug=False, num_devices=N_CORES
    )

    # ---- DRAM I/O ----
    qT_d = nc.dram_tensor("qT16", [EMBED, SQ], f16, kind="ExternalInput").ap()
    kT_d = nc.dram_tensor("kT16", [EMBED, SEQ], f16, kind="ExternalInput").ap()
    vT_d = nc.dram_tensor("vT16", [EMBED, SEQ], f16, kind="ExternalInput").ap()
    wq2_d = nc.dram_tensor("wq2", [128, 128], f16, kind="ExternalInput").ap()
    wk2_d = nc.dram_tensor("wk2", [128, 128], f16, kind="ExternalInput").ap()
    wv2_d = nc.dram_tensor("wv2", [128, 128], f16, kind="ExternalInput").ap()
    pkT_d = nc.dram_tensor("pkT", [HD, NPERS], f16, kind="ExternalInput").ap()
    pv_d = nc.dram_tensor("pv", [NPERS, HD], f16, kind="ExternalInput").ap()
    wpcol_d = nc.dram_tensor(
        "wpcol", [128, HEADS * 4], f32, kind="ExternalInput"
    ).ap()
    cgcol_d = nc.dram_tensor("cgcol", [128, 2], f32, kind="ExternalInput").ap()
    w2p_d = {
        nm: nc.dram_tensor(nm, [128, 256], f32, kind="ExternalInput").ap()
        for nm in ("w2pA", "w2pB")
    }
    dist_d = nc.dram_tensor("dist", [128, DIST_W], f16, kind="ExternalInput").ap()
    fcwT_d = nc.dram_tensor("fcwT", [EMBED, EMBED], f16, kind="ExternalInput").ap()
    fcb_d = nc.dram_tensor("fcb", [1, EMBED], f16, kind="ExternalInput").ap()
    out_d = nc.dram_tensor("out", [SQ, EMBED], f32, kind="ExternalOutput").ap()

    def ecopy(eng, out, in_):
        if eng is nc.scalar:
            eng.copy(out, in_)
        else:
            eng.tensor_copy(out, in_)

    with tile.TileContext(nc) as tc, ExitStack() as ctx:
        const_pool = ctx.enter_context(tc.tile_pool(name="const", bufs=1))
        xT_pool = ctx.enter_context(tc.tile_pool(name="xT", bufs=1))
        vs_pool = ctx.enter_context(tc.tile_pool(name="vs", bufs=1))
        qt_pool = ctx.enter_context(tc.tile_pool(name="qt", bufs=2))
        xrq_pool = ctx.enter_context(tc.tile_pool(name="xrq", bufs=4))
        xrk_pool = ctx.enter_context(tc.tile_pool(name="xrk", bufs=4))
        exp_pool = ctx.enter_context(tc.tile_pool(name="expp", bufs=2))
        smx_pool = ctx.enter_context(tc.tile_pool(name="smx", bufs=3))
        aT_pool = ctx.enter_context(tc.tile_pool(name="aTp", bufs=1))
        at_pool = ctx.enter_context(tc.tile_pool(name="atp", bufs=1))
        os_pool = ctx.enter_context(tc.tile_pool(name="osp", bufs=1))

        # PSUM pools: en 4 + m2 2 + aux 2 = 8 banks (proj/fc share m2)
        en_ps = ctx.enter_context(tc.tile_pool(name="en_ps", bufs=4, space="PSUM"))
        m2_ps = ctx.enter_context(tc.tile_pool(name="m2_ps", bufs=2, space="PSUM"))
        aux_ps = ctx.enter_context(tc.tile_pool(name="aux_ps", bufs=2, space="PSUM"))

        # ---- constants ----
        def cload(ap_d, shape, dtype, nm):
            t = const_pool.tile(shape, dtype, tag=nm, name=nm)
            nc.sync.dma_start(t[:], ap_d)
            return t

        wq2t = cload(wq2_d, [128, 128], f16, "wq2t")
        wk2t = cload(wk2_d, [128, 128], f16, "wk2t")
        wv2t = cload(wv2_d, [128, 128], f16, "wv2t")
        pkTt = cload(pkT_d, [HD, NPERS], f16, "pkTt")
        pvt = cload(pv_d, [NPERS, HD], f16, "pvt")
        wpcolt = cload(wpcol_d, [128, HEADS * 4], f32, "wpcolt")
        cgcolt = cload(cgcol_d, [128, 2], f32, "cgcolt")
        w2pt = {nm: cload(d, [128, 256], f32, nm) for nm, d in w2p_d.items()}
        ones1 = const_pool.tile([1, 128], f16, tag="ones1", name="ones1")
        nc.vector.memset(ones1[:], 1.0)

        # ---- persistent activation tiles ----
        qPT = [
            xT_pool.tile([128, SQ], f16, tag=f"qPT{p}", name=f"qPT{p}")
            for p in range(4)
        ]
        kPT = [
            xT_pool.tile([128, KT], f16, tag=f"kPT{p}", name=f"kPT{p}")
            for p in range(4)
        ]
        vS = [
            vs_pool.tile([128, EMBED], f16, tag=f"vS{j}", name=f"vS{j}")
            for j in range(NKJ)
        ]
        vSp = vs_pool.tile([NPERS, EMBED], f16, tag="vSp", name="vSp")

        for p in range(4):
            nc.vector.tensor_copy(kPT[p][0:HD, SEQ:KT], pkTt[:])
            nc.vector.tensor_copy(kPT[p][HD:128, SEQ:KT], pkTt[:])
        for h in range(HEADS):
            nc.vector.tensor_copy(vSp[:, HD * h : HD * (h + 1)], pvt[:])

        # ---- raw activation DMAs (k first: block 0 needs it first) ----
        rTk, rTq = [], []
        for p in range(4):
            t = xrk_pool.tile([128, SEQ], f16, tag="xk", name=f"rTk{p}")
            nc.sync.dma_start(t[:], kT_d[128 * p : 128 * (p + 1), :])
            rTk.append(t)
        def dma_q(c):
            for p in range(4):
                t = xrq_pool.tile([128, 512], f16, tag="xq", name=f"rTq{p}")
                nc.sync.dma_start(
                    t[:], qT_d[128 * p : 128 * (p + 1), 512 * c : 512 * (c + 1)]
                )
                rTq.append(t)

        dma_q(0)

        def emit_qproj(c):
            for p in range(4):
                ps = m2_ps.tile([128, 512], f32, tag="m2", name="psq")
                nc.tensor.matmul(
                    ps[:],
                    lhsT=wq2t[:],
                    rhs=rTq[4 * c + p][:],
                    start=True,
                    stop=True,
                )
                nc.scalar.copy(qPT[p][:, 512 * c : 512 * (c + 1)], ps[:])

        def emit_kproj(c):
            for p in range(4):
                ps = m2_ps.tile([128, 512], f32, tag="m2", name="psk")
                nc.tensor.matmul(
                    ps[:],
                    lhsT=wk2t[:],
                    rhs=rTk[p][:, 512 * c : 512 * (c + 1)],
                    start=True,
                    stop=True,
                )
                nc.scalar.copy(kPT[p][:, 512 * c : 512 * (c + 1)], ps[:])

        rTv = {}

        def dma_v(p):
            t = xrk_pool.tile([128, SEQ], f16, tag="xk", name=f"rTv{p}")
            nc.sync.dma_start(t[:], vT_d[128 * p : 128 * (p + 1), :])
            rTv[p] = t

        def emit_vproj(p):
            # v: project and transpose back to natural [seq, (h,d)] in one
            # matmul (vT chunk stationary against block-diag weights).
            for j in range(NKJ):
                ps = m2_ps.tile([128, 512], f32, tag="m2", name="psv")
                nc.tensor.matmul(
                    ps[:, 0:128],
                    lhsT=rTv[p][:, 128 * j : 128 * (j + 1)],
                    rhs=wv2t[:],
                    start=True,
                    stop=True,
                )
                nc.scalar.copy(vS[j][:, 128 * p : 128 * (p + 1)], ps[:, 0:128])

        # q-proj chunk 0 covers groups 0-1's q~ columns
        emit_qproj(0)

        # late-needed constants: issued after the activation DMAs so the
        # rT loads aren't queued behind ~2 MB of tables
        distt = cload(dist_d, [128, DIST_W], f16, "distt")
        fcw = []
        for cc in range(HEADS):
            t = const_pool.tile([HD, EMBED], f16, tag=f"fcw{cc}", name=f"fcw{cc}")
            nc.sync.dma_start(t[:], fcwT_d[HD * cc : HD * (cc + 1), :])
            fcw.append(t)
        fcbt = cload(fcb_d, [1, EMBED], f16, "fcbt")

        # q~: per pair, 8 out-head-scaled copies of the group's q'T columns
        # (double-buffered; next group's tiles built during current group)
        def build_qtil(qg, p):
            t = qt_pool.tile(
                [128, HEADS * QG], f16, tag=f"qt{p}", name=f"qt{p}g{qg}"
            )
            qv = t.rearrange("p (b o a q) -> p b o a q", b=QBPG, o=2, a=4, q=QB)
            pv_in = qPT[p][:, qg * QG : (qg + 1) * QG].rearrange(
                "p (b q) -> p b q", b=QBPG, q=QB
            )
            for g in range(HEADS):
                nc.vector.tensor_scalar_mul(
                    qv[:, :, g // 4, g % 4, :],
                    pv_in[:],
                    wpcolt[:, 4 * g + p : 4 * g + p + 1],
                )
            return t

        qtil_next = [build_qtil(0, p) for p in range(4)]

        # ---- main attention loop ----
        for qg in range(NQG):
            qtil = qtil_next
            qtil_next = [None] * 4

            aT = aT_pool.tile([128, NKJ * HEADS * QG], f16, tag="aT", name="aT")
            aTv = aT.rearrange("p (j g q) -> p j g q", j=NKJ, g=HEADS, q=QG)
            aTp = aT_pool.tile([NPERS, HEADS * QG], f16, tag="aTp", name="aTp")
            aTpv = aTp.rearrange("p (g q) -> p g q", g=HEADS, q=QG)

            def emit_mix2(qb_i, ex_s, m2b):
                # mix2 transposed: A'^T[k, (g, qi)], two k-subchunks per tile
                for jj in range(NKJ // 2):
                    m2 = m2_ps.tile([128, 512], f32, tag="m2", name="m2")
                    for j2 in range(2):
                        j = 2 * jj + j2
                        for ob in ("A", "B"):
                            nc.tensor.matmul(
                                m2[:, 256 * j2 : 256 * (j2 + 1)],
                                lhsT=ex_s[ob][:, 128 * j : 128 * (j + 1)],
                                rhs=m2b[ob][:],
                                start=(ob == "A"),
                                stop=(ob == "B"),
                            )
                    m2v = m2.rearrange("p (j g q) -> p j g q", j=2, g=HEADS, q=QB)
                    eng = nc.vector if jj % 2 == 0 else nc.scalar
                    ecopy(
                        eng,
                        aTv[:, 2 * jj : 2 * (jj + 1), :, qb_i * QB : (qb_i + 1) * QB],
                        m2v[:],
                    )
                # persistent k rows
                m2 = m2_ps.tile([128, 512], f32, tag="m2", name="m2p")
                for ob in ("A", "B"):
                    nc.tensor.matmul(
                        m2[0:NPERS, 0:256],
                        lhsT=ex_s[ob][:, SEQ:KT],
                        rhs=m2b[ob][:],
                        start=(ob == "A"),
                        stop=(ob == "B"),
                    )
                m2pv = m2[0:NPERS, 0:256].rearrange(
                    "p (g q) -> p g q", g=HEADS, q=QB
                )
                nc.vector.tensor_copy(
                    aTpv[:, :, qb_i * QB : (qb_i + 1) * QB], m2pv[:]
                )

            # software-pipelined over blocks: mix2 of block b is emitted
            # after the scores of block b+1, so the PE streams b+1's scores
            # while the DVE finishes b's denominator chain (reduce ->
            # reciprocal -> m2b build) instead of stalling on it.
            pend = None
            for qb_i in range(QBPG):
                b = qg * QBPG + qb_i
                qc0 = b * QB  # core-local query offset of this block

                # fused scores + W_pre mix + alibi, then exp
                # bank A: out-heads 0-3 (rows 32*ga + qi), bank B: 4-7
                ex_s = {}
                acc = smx_pool.tile([128, 16], f32, tag="acc", name="acc")
                for ob in ("A", "B"):
                    ex_s[ob] = exp_pool.tile(
                        [128, KT], bf16, tag=f"exp{ob}", name=f"ex_s{ob}"
                    )
                for c in range(NKC + 1):
                    if b == 0 and c < NKC:
                        emit_kproj(c)
                    k0 = c * KC
                    w = KC if c < NKC else NPERS
                    has_alibi = c < NKC
                    for oi, ob in enumerate(("A", "B")):
                        en = en_ps.tile([128, KC], f32, tag="en", name="en")
                        for p in range(4):
                            c0 = qb_i * 256 + oi * 128
                            lhs = qtil[p][:, c0 : c0 + 128]
                            nc.tensor.matmul(
                                en[:, 0:w],
                                lhsT=lhs,
                                rhs=kPT[p][:, k0 : k0 + w],
                                start=(p == 0),
                                stop=(p == 3),
                            )
                        if has_alibi:
                            # en += c_g * dist, on DVE (frees PE cycles)
                            u0 = SEQ + k0 - qc0 - TOFS
                            nc.vector.scalar_tensor_tensor(
                                en[:, 0:w],
                                distt[:, u0 : u0 + w],
                                cgcolt[:, oi : oi + 1],
                                en[:, 0:w],
                                op0=mybir.AluOpType.mult,
                                op1=mybir.AluOpType.add,
                            )
                        nc.scalar.activation(
                            ex_s[ob][:, k0 : k0 + w],
                            en[:, 0:w],
                            EXP,
                            scale=SCALE,
                            accum_out=acc[:, 2 * c + oi : 2 * c + oi + 1],
                        )

                # softmax denominators; fold 1/sum into the mix2 matrices
                m2b = {}
                for oi, ob in enumerate(("A", "B")):
                    sm = smx_pool.tile([128, 2], f32, tag=f"sm{ob}", name=f"sm{ob}")
                    nc.vector.reduce_sum(
                        sm[:, 0:1],
                        acc.rearrange("p (c o) -> p c o", o=2)[:, 0 : NKC + 1, oi],
                        axis=X,
                    )
                    rc = smx_pool.tile([128, 1], f32, tag=f"rc{ob}", name=f"rc{ob}")
                    nc.vector.reciprocal(rc[:], sm[:, 0:1])
                    m2b[ob] = smx_pool.tile(
                        [128, 256], bf16, tag=f"m2b{ob}", name=f"m2b{ob}"
                    )
                    nc.vector.tensor_scalar_mul(
                        m2b[ob][:], w2pt[f"w2p{ob}"][:], rc[:, 0:1]
                    )

                if pend is not None:
                    emit_mix2(*pend)
                pend = (qb_i, ex_s, m2b)

                # interleaved late work (PE: proj; DVE: next group's q~)
                if b == 0:
                    dma_q(1)
                    emit_qproj(1)
                if 1 <= b <= 4:
                    dma_v(b - 1)
                if 2 <= b <= 5:
                    emit_vproj(b - 2)
                if 2 <= qb_i <= 5 and qg + 1 < NQG:
                    qtil_next[qb_i - 2] = build_qtil(qg + 1, qb_i - 2)
            emit_mix2(*pend)

            # ---- A@V for the group: attn^T[d, q] per out-head g ----
            # two av accumulation regions per PSUM bank: with bufs=2 the
            # copy-out of head g's result only gates head g+4's matmuls
            # (not g+2's), hiding the aT/at_s copy backlog on DVE/ACT
            at_s = []
            auxt = None
            for g in range(HEADS):
                if g % 2 == 0:
                    auxt = aux_ps.tile([128, 2 * QG], f32, tag="aux", name="av2")
                av = auxt[:, QG * (g % 2) : QG * (g % 2 + 1)]
                for j in range(NKJ):
                    nc.tensor.matmul(
                        av[0:HD, :],
                        lhsT=vS[j][:, HD * g : HD * (g + 1)],
                        rhs=aTv[:, j, g, :],
                        start=(j == 0),
                        stop=False,
                    )
                nc.tensor.matmul(
                    av[0:HD, :],
                    lhsT=vSp[:, HD * g : HD * (g + 1)],
                    rhs=aTpv[:, g, :],
                    start=False,
                    stop=True,
                )
                ats = at_pool.tile([HD, QG], f16, tag=f"ats{g}", name=f"ats{g}")
                eng = nc.scalar if g % 2 == 0 else nc.vector
                ecopy(eng, ats[:], av[0:HD, :])
                at_s.append(ats)

            # ---- fc_out: 8 accumulated K=64 matmuls + rank-1 bias ----
            # both 128-row output halves accumulate interleaved per head, so
            # the first fc matmul only waits on head 0's at_s copy (not 7's)
            fps = [
                m2_ps.tile([128, EMBED], f32, tag="m2", name=f"fp{sub}")
                for sub in (0, 1)
            ]
            for g in range(HEADS):
                for sub in (0, 1):
                    nc.tensor.matmul(
                        fps[sub][:],
                        lhsT=at_s[g][:, 128 * sub : 128 * (sub + 1)],
                        rhs=fcw[g][:],
                        start=(g == 0),
                        stop=False,
                    )
            for sub in (0, 1):
                nc.tensor.matmul(
                    fps[sub][:], lhsT=ones1[:], rhs=fcbt[:], start=False,
                    stop=True,
                )
                o_s = os_pool.tile([128, EMBED], f32, tag="os", name="o_s")
                nc.vector.tensor_copy(o_s[:], fps[sub][:])
                q_row = qg * QG + sub * 128
                nc.sync.dma_start(out_d[q_row : q_row + 128, :], o_s[:])

    nc.compile()
    return nc


def _get_nc():
    if "nc" not in _CACHED:
        _CACHED["nc"] = build_bass()
    return _CACHED["nc"]


def kernel(
    values,
    keys,
    queries,
    mask,
    Wv,
    Wk,
    Wq,
    W_pre,
    W_post,
    p_keys,
    p_values,
    fc_w,
    fc_b,
):
    """Full-input entry point. mask is all-True per the problem spec
    (fill: ones) and is therefore not consumed on-device."""
    from concourse.bass_utils import run_bass_kernel_spmd

    qT = np.asarray(queries, np.float32).astype(np.float16).transpose(0, 2, 1)
    kT = np.asarray(keys, np.float32).astype(np.float16).transpose(0, 2, 1)
    vT = np.asarray(values, np.float32).astype(np.float16).transpose(0, 2, 1)
    consts = _host_consts(
        np.asarray(Wv, np.float32),
        np.asarray(Wk, np.float32),
        np.asarray(Wq, np.float32),
        np.asarray(W_pre, np.float32),
        np.asarray(W_post, np.float32),
        np.asarray(p_keys, np.float32),
        np.asarray(p_values, np.float32),
        np.asarray(fc_w, np.float32),
        np.asarray(fc_b, np.float32),
    )

    nc = _get_nc()
    in_maps = []
    for core in range(N_CORES):
        n, half = core // 2, core % 2
        qbase = half * SQ
        m = {
            "qT16": np.ascontiguousarray(qT[n, :, qbase : qbase + SQ]),
            "kT16": np.ascontiguousarray(kT[n]),
            "vT16": np.ascontiguousarray(vT[n]),
            "dist": _dist_table(qbase),
        }
        m.update(consts)
        in_maps.append(m)

    global _last_in_maps
    _last_in_maps = in_maps
    res = run_bass_kernel_spmd(nc, in_maps, core_ids=list(range(N_CORES)))
    out = np.empty((N_BATCH, SEQ, EMBED), np.float32)
    for core in range(N_CORES):
        n, half = core // 2, core % 2
        out[n, half * SQ : (half + 1) * SQ, :] = res.results[core]["out"]
    return out


# revision 20
# speedup vs baseline: 1.9130x; 1.0330x over previous
"""TRN2 Bass/Tile kernel for NeoTCNAttention (talking-heads attention with
ALiBi + persistent memory), SPMD over 8 NeuronCores.

Sharding: data-parallel over batch N=4 x 2 halves of the query axis
(each core: one batch element, 1024 query positions, full keys/values).
No collectives: every core computes a disjoint slab of the output.

Per-core pipeline (all matmuls are standard full-array ops):
  - activations arrive transposed/fp16 (x^T pair tiles [128=(2 heads x 64
    dims), seq]); Q/K/V projections use host-built block-diag(W^T, W^T)
    weights, one matmul per 512-wide chunk.  The k projection is woven
    into block 0's score-chunk loop and the v projection is spread over
    blocks 1-5 so the PE never idles in a projection prologue.
  - W_pre (pre-softmax talking heads) is folded into the score matmuls:
    q~[(h,d), (g, qi)] = W_pre[g, h] * q'^T, so for each output bank
    (4 g-heads x 32 q rows) the scores+mix are 4 accumulated pair matmuls;
    ALiBi lands as a DVE scalar_tensor_tensor against a distance table
    (slopes folded through W_pre into per-g scalars c_g).
  - exp runs on the scalar engine straight out of PSUM with accumulated
    row sums; softmax normalization is folded into the post-softmax
    talking-heads matrix (per-row 1/sum scaling).
  - q~ tiles are double-buffered: the next group's W_pre-scaled copies are
    built on the DVE during blocks 2-5 of the current group.
  - post-softmax talking heads run as transposing matmuls (exp tile as
    the stationary operand) producing A'^T [k, (g, q)] directly, which
    feeds A@V with v' columns as stationary weights.
  - fc_out accumulates 8 K=64 matmuls with the bias folded in as a
    rank-1 (ones x fc_b) matmul.
"""

import math

import numpy as np

# ---- problem constants (hardcoded per spec) ----
N_BATCH = 4
SEQ = 2048
EMBED = 512
HEADS = 8
HD = 64
NPERS = 16
KT = SEQ + NPERS  # 2064
ALIBI_ALPHA = 1.25
START_I = 1
N_CORES = 8
SQ = 1024  # query positions per core
SCALE = 1.0 / math.sqrt(EMBED)

QB = 32            # query positions per block (bank rows = 4 g x 32 qi)
NQB = SQ // QB     # 32 blocks
QG = 256           # query positions per A@V group
NQG = SQ // QG     # 4 groups
QBPG = QG // QB    # 8 blocks per group
KC = 512           # k chunk width for scores/exp
NKC = SEQ // KC    # 4 (persistent 16 handled separately)
NKJ = SEQ // 128   # 16 mix2/A@V k-subchunks (+1 partial of NPERS)
TOFS = 1024        # distance-table column offset (u >= 1056 always)
DIST_W = SEQ + KT - TOFS  # 3088 stored distance-table width

_CACHED = {}
_last_in_maps = None


def _host_consts(Wv, Wk, Wq, W_pre, W_post, p_keys, p_values, fc_w, fc_b):
    """Derived constant tensors shipped to the device (layout prep only)."""
    f16 = np.float16
    Z = np.zeros((HD, HD), np.float32)
    c = {}
    c["wq2"] = np.block([[Wq.T, Z], [Z, Wq.T]]).astype(f16)  # [128, 128]
    c["wk2"] = np.block([[Wk.T, Z], [Z, Wk.T]]).astype(f16)
    c["wv2"] = np.block([[Wv.T, Z], [Z, Wv.T]]).astype(f16)
    c["pkT"] = np.ascontiguousarray(p_keys[:, 0, :].T).astype(f16)   # [64, 16]
    c["pv"] = np.ascontiguousarray(p_values[:, 0, :]).astype(f16)    # [16, 64]

    # W_pre scaling columns for the q~ build: col 4*g + p scales pair p's
    # rows (head 2p in partitions 0-63, head 2p+1 in 64-127) for out-head g.
    wpcol = np.zeros((128, HEADS * 4), np.float32)
    for g in range(HEADS):
        for p in range(4):
            wpcol[0:HD, 4 * g + p] = W_pre[g, 2 * p]
            wpcol[HD:128, 4 * g + p] = W_pre[g, 2 * p + 1]
    c["wpcol"] = wpcol

    slopes = 2.0 ** (
        -ALIBI_ALPHA * (np.arange(1, HEADS + 1, dtype=np.float64) + START_I)
    )
    cg_vec = -(W_pre.astype(np.float64) @ slopes)  # per-out-head slope mix

    cgcol = np.zeros((128, 2), np.float32)
    for oi in range(2):
        for ga in range(4):
            cgcol[32 * ga : 32 * (ga + 1), oi] = cg_vec[4 * oi + ga]
    c["cgcol"] = cgcol

    idx = np.arange(QB)
    for ob, gofs in (("A", 0), ("B", 4)):
        w2p = np.zeros((128, 256), np.float32)
        for ga in range(4):
            for g2 in range(HEADS):
                w2p[32 * ga + idx, 32 * g2 + idx] = W_post[g2, gofs + ga]
        c[f"w2p{ob}"] = w2p.astype(np.float32)

    c["fcwT"] = np.ascontiguousarray(fc_w.T).astype(f16)  # [512, 512] rows=(g,d)
    c["fcb"] = fc_b.reshape(1, EMBED).astype(f16)
    return c


def _dist_table(qbase: int) -> np.ndarray:
    """T[32*ga + qi, u - TOFS] = |qbase + qi - (u - SEQ)| as fp16 (rows
    replicated over the 4 out-heads of a bank; exact for ints <= 2048)."""
    u = np.arange(TOFS, TOFS + DIST_W)
    qi = np.arange(QB)
    t = np.abs(qbase + qi[:, None] - (u[None, :] - SEQ)).astype(np.float16)
    return np.tile(t, (4, 1))


def build_bass():
    import concourse.mybir as mybir
    import concourse.tile as tile
    from concourse import bacc
    from contextlib import ExitStack

    f32 = mybir.dt.float32
    f16 = mybir.dt.float16
    bf16 = mybir.dt.bfloat16
    EXP = mybir.ActivationFunctionType.Exp
    X = mybir.AxisListType.X

    nc = bacc.Bacc(
        "TRN2", target_bir_lowering=False, debug=False, num_devices=N_CORES
    )

    # ---- DRAM I/O ----
    qT_d = nc.dram_tensor("qT16", [EMBED, SQ], f16, kind="ExternalInput").ap()
    kT_d = nc.dram_tensor("kT16", [EMBED, SEQ], f16, kind="ExternalInput").ap()
    vT_d = nc.dram_tensor("vT16", [EMBED, SEQ], f16, kind="ExternalInput").ap()
    wq2_d = nc.dram_tensor("wq2", [128, 128], f16, kind="ExternalInput").ap()
    wk2_d = nc.dram_tensor("wk2", [128, 128], f16, kind="ExternalInput").ap()
    wv2_d = nc.dram_tensor("wv2", [128, 128], f16, kind="ExternalInput").ap()
    pkT_d = nc.dram_tensor("pkT", [HD, NPERS], f16, kind="ExternalInput").ap()
    pv_d = nc.dram_tensor("pv", [NPERS, HD], f16, kind="ExternalInput").ap()
    wpcol_d = nc.dram_tensor(
        "wpcol", [128, HEADS * 4], f32, kind="ExternalInput"
    ).ap()
    cgcol_d = nc.dram_tensor("cgcol", [128, 2], f32, kind="ExternalInput").ap()
    w2p_d = {
        nm: nc.dram_tensor(nm, [128, 256], f32, kind="ExternalInput").ap()
        for nm in ("w2pA", "w2pB")
    }
    dist_d = nc.dram_tensor("dist", [128, DIST_W], f16, kind="ExternalInput").ap()
    fcwT_d = nc.dram_tensor("fcwT", [EMBED, EMBED], f16, kind="ExternalInput").ap()
    fcb_d = nc.dram_tensor("fcb", [1, EMBED], f16, kind="ExternalInput").ap()
    out_d = nc.dram_tensor("out", [SQ, EMBED], f32, kind="ExternalOutput").ap()

    def ecopy(eng, out, in_):
        if eng is nc.scalar:
            eng.copy(out, in_)
        else:
            eng.tensor_copy(out, in_)

    with tile.TileContext(nc) as tc, ExitStack() as ctx:
        const_pool = ctx.enter_context(tc.tile_pool(name="const", bufs=1))
        xT_pool = ctx.enter_context(tc.tile_pool(name="xT", bufs=1))
        vs_pool = ctx.enter_context(tc.tile_pool(name="vs", bufs=1))
        qt_pool = ctx.enter_context(tc.tile_pool(name="qt", bufs=1))
        xrq_pool = ctx.enter_context(tc.tile_pool(name="xrq", bufs=4))
        xrk_pool = ctx.enter_context(tc.tile_pool(name="xrk", bufs=4))
        exp_pool = ctx.enter_context(tc.tile_pool(name="expp", bufs=3))
        smx_pool = ctx.enter_context(tc.tile_pool(name="smx", bufs=3))
        aT_pool = ctx.enter_context(tc.tile_pool(name="aTp", bufs=1))
        at_pool = ctx.enter_context(tc.tile_pool(name="atp", bufs=2))
        os_pool = ctx.enter_context(tc.tile_pool(name="osp", bufs=2))

        # PSUM pools: en 4 + m2 2 + aux 2 = 8 banks (proj/fc share m2)
        en_ps = ctx.enter_context(tc.tile_pool(name="en_ps", bufs=4, space="PSUM"))
        m2_ps = ctx.enter_context(tc.tile_pool(name="m2_ps", bufs=2, space="PSUM"))
        aux_ps = ctx.enter_context(tc.tile_pool(name="aux_ps", bufs=2, space="PSUM"))

        # ---- constants ----
        def cload(ap_d, shape, dtype, nm):
            t = const_pool.tile(shape, dtype, tag=nm, name=nm)
            nc.sync.dma_start(t[:], ap_d)
            return t

        wq2t = cload(wq2_d, [128, 128], f16, "wq2t")
        wk2t = cload(wk2_d, [128, 128], f16, "wk2t")
        wv2t = cload(wv2_d, [128, 128], f16, "wv2t")
        pkTt = cload(pkT_d, [HD, NPERS], f16, "pkTt")
        pvt = cload(pv_d, [NPERS, HD], f16, "pvt")
        wpcolt = cload(wpcol_d, [128, HEADS * 4], f32, "wpcolt")
        cgcolt = cload(cgcol_d, [128, 2], f32, "cgcolt")
        w2pt = {nm: cload(d, [128, 256], f32, nm) for nm, d in w2p_d.items()}
        ones1 = const_pool.tile([1, 128], f16, tag="ones1", name="ones1")
        nc.vector.memset(ones1[:], 1.0)

        # ---- persistent activation tiles ----
        qPT = [
            xT_pool.tile([128, SQ], f16, tag=f"qPT{p}", name=f"qPT{p}")
            for p in range(4)
        ]
        kPT = [
            xT_pool.tile([128, KT], f16, tag=f"kPT{p}", name=f"kPT{p}")
            for p in range(4)
        ]
        vS = [
            vs_pool.tile([128, EMBED], f16, tag=f"vS{j}", name=f"vS{j}")
            for j in range(NKJ)
        ]
        vSp = vs_pool.tile([NPERS, EMBED], f16, tag="vSp", name="vSp")

        for p in range(4):
            nc.vector.tensor_copy(kPT[p][0:HD, SEQ:KT], pkTt[:])
            nc.vector.tensor_copy(kPT[p][HD:128, SEQ:KT], pkTt[:])
        for h in range(HEADS):
            nc.vector.tensor_copy(vSp[:, HD * h : HD * (h + 1)], pvt[:])

        # ---- raw activation DMAs (k first: block 0 needs it first) ----
        rTk, rTq = [], []
        for p in range(4):
            t = xrk_pool.tile([128, SEQ], f16, tag="xk", name=f"rTk{p}")
            nc.sync.dma_start(t[:], kT_d[128 * p : 128 * (p + 1), :])
            rTk.append(t)
        def dma_q(c):
            for p in range(4):
                t = xrq_pool.tile([128, 512], f16, tag="xq", name=f"rTq{p}")
                nc.sync.dma_start(
                    t[:], qT_d[128 * p : 128 * (p + 1), 512 * c : 512 * (c + 1)]
                )
                rTq.append(t)

        dma_q(0)

        def emit_qproj(c):
            for p in range(4):
                ps = m2_ps.tile([128, 512], f32, tag="m2", name="psq")
                nc.tensor.matmul(
                    ps[:],
                    lhsT=wq2t[:],
                    rhs=rTq[4 * c + p][:],
                    start=True,
                    stop=True,
                )
                nc.scalar.copy(qPT[p][:, 512 * c : 512 * (c + 1)], ps[:])

        def emit_kproj(c):
            for p in range(4):
                ps = m2_ps.tile([128, 512], f32, tag="m2", name="psk")
                nc.tensor.matmul(
                    ps[:],
                    lhsT=wk2t[:],
                    rhs=rTk[p][:, 512 * c : 512 * (c + 1)],
                    start=True,
                    stop=True,
                )
                nc.scalar.copy(kPT[p][:, 512 * c : 512 * (c + 1)], ps[:])

        rTv = {}

        def dma_v(p):
            t = xrk_pool.tile([128, SEQ], f16, tag="xk", name=f"rTv{p}")
            nc.sync.dma_start(t[:], vT_d[128 * p : 128 * (p + 1), :])
            rTv[p] = t

        def emit_vproj(p):
            # v: project and transpose back to natural [seq, (h,d)] in one
            # matmul (vT chunk stationary against block-diag weights).
            for j in range(NKJ):
                ps = m2_ps.tile([128, 512], f32, tag="m2", name="psv")
                nc.tensor.matmul(
                    ps[:, 0:128],
                    lhsT=rTv[p][:, 128 * j : 128 * (j + 1)],
                    rhs=wv2t[:],
                    start=True,
                    stop=True,
                )
                nc.scalar.copy(vS[j][:, 128 * p : 128 * (p + 1)], ps[:, 0:128])

        # q-proj chunk 0 covers groups 0-1's q~ columns
        emit_qproj(0)

        # late-needed constants: issued after the activation DMAs so the
        # rT loads aren't queued behind ~2 MB of tables
        distt = cload(dist_d, [128, DIST_W], f16, "distt")
        fcw = []
        for cc in range(HEADS):
            t = const_pool.tile([HD, EMBED], f16, tag=f"fcw{cc}", name=f"fcw{cc}")
            nc.sync.dma_start(t[:], fcwT_d[HD * cc : HD * (cc + 1), :])
            fcw.append(t)
        fcbt = cload(fcb_d, [1, EMBED], f16, "fcbt")

        # q~: per pair, 8 out-head-scaled copies of the group's q'T columns
        # (double-buffered; next group's tiles built during current group)
        def build_qtil(qg, p):
            t = qt_pool.tile(
                [128, HEADS * QG], f16, tag=f"qt{p}", name=f"qt{p}g{qg}"
            )
            qv = t.rearrange("p (b o a q) -> p b o a q", b=QBPG, o=2, a=4, q=QB)
            pv_in = qPT[p][:, qg * QG : (qg + 1) * QG].rearrange(
                "p (b q) -> p b q", b=QBPG, q=QB
            )
            for g in range(HEADS):
                nc.vector.tensor_scalar_mul(
                    qv[:, :, g // 4, g % 4, :],
                    pv_in[:],
                    wpcolt[:, 4 * g + p : 4 * g + p + 1],
                )
            return t

        qtil = [build_qtil(0, p) for p in range(4)]

        # ---- main attention loop ----
        for qg in range(NQG):

            aT = aT_pool.tile([128, NKJ * HEADS * QG], f16, tag="aT", name="aT")
            aTv = aT.rearrange("p (j g q) -> p j g q", j=NKJ, g=HEADS, q=QG)
            aTp = aT_pool.tile([NPERS, HEADS * QG], f16, tag="aTp", name="aTp")
            aTpv = aTp.rearrange("p (g q) -> p g q", g=HEADS, q=QG)

            def emit_mix2(qb_i, ex_s, m2b):
                # mix2 transposed: A'^T[k, (g, qi)], two k-subchunks per tile
                for jj in range(NKJ // 2):
                    m2 = m2_ps.tile([128, 512], f32, tag="m2", name="m2")
                    for j2 in range(2):
                        j = 2 * jj + j2
                        for ob in ("A", "B"):
                            nc.tensor.matmul(
                                m2[:, 256 * j2 : 256 * (j2 + 1)],
                                lhsT=ex_s[ob][:, 128 * j : 128 * (j + 1)],
                                rhs=m2b[ob][:],
                                start=(ob == "A"),
                                stop=(ob == "B"),
                            )
                    m2v = m2.rearrange("p (j g q) -> p j g q", j=2, g=HEADS, q=QB)
                    eng = nc.vector if jj % 2 == 0 else nc.scalar
                    ecopy(
                        eng,
                        aTv[:, 2 * jj : 2 * (jj + 1), :, qb_i * QB : (qb_i + 1) * QB],
                        m2v[:],
                    )
                # persistent k rows
                m2 = m2_ps.tile([128, 512], f32, tag="m2", name="m2p")
                for ob in ("A", "B"):
                    nc.tensor.matmul(
                        m2[0:NPERS, 0:256],
                        lhsT=ex_s[ob][:, SEQ:KT],
                        rhs=m2b[ob][:],
                        start=(ob == "A"),
                        stop=(ob == "B"),
                    )
                m2pv = m2[0:NPERS, 0:256].rearrange(
                    "p (g q) -> p g q", g=HEADS, q=QB
                )
                nc.vector.tensor_copy(
                    aTpv[:, :, qb_i * QB : (qb_i + 1) * QB], m2pv[:]
                )

            # software-pipelined over blocks: mix2 of block b is emitted
            # after the scores of block b+1, so the PE streams b+1's scores
            # while the DVE finishes b's denominator chain (reduce ->
            # reciprocal -> m2b build) instead of stalling on it.
            pend = None
            for qb_i in range(QBPG):
                b = qg * QBPG + qb_i
                qc0 = b * QB  # core-local query offset of this block

                # fused scores + W_pre mix + alibi, then exp
                # bank A: out-heads 0-3 (rows 32*ga + qi), bank B: 4-7
                ex_s = {}
                acc = smx_pool.tile([128, 16], f32, tag="acc", name="acc")
                for ob in ("A", "B"):
                    ex_s[ob] = exp_pool.tile(
                        [128, KT], bf16, tag=f"exp{ob}", name=f"ex_s{ob}"
                    )
                for c in range(NKC + 1):
                    if b == 0 and c < NKC:
                        emit_kproj(c)
                    k0 = c * KC
                    w = KC if c < NKC else NPERS
                    has_alibi = c < NKC
                    for oi, ob in enumerate(("A", "B")):
                        en = en_ps.tile([128, KC], f32, tag="en", name="en")
                        for p in range(4):
                            c0 = qb_i * 256 + oi * 128
                            lhs = qtil[p][:, c0 : c0 + 128]
                            nc.tensor.matmul(
                                en[:, 0:w],
                                lhsT=lhs,
                                rhs=kPT[p][:, k0 : k0 + w],
                                start=(p == 0),
                                stop=(p == 3),
                            )
                        if has_alibi:
                            # en += c_g * dist, on DVE (frees PE cycles)
                            u0 = SEQ + k0 - qc0 - TOFS
                            nc.vector.scalar_tensor_tensor(
                                en[:, 0:w],
                                distt[:, u0 : u0 + w],
                                cgcolt[:, oi : oi + 1],
                                en[:, 0:w],
                                op0=mybir.AluOpType.mult,
                                op1=mybir.AluOpType.add,
                            )
                        nc.scalar.activation(
                            ex_s[ob][:, k0 : k0 + w],
                            en[:, 0:w],
                            EXP,
                            scale=SCALE,
                            accum_out=acc[:, 2 * c + oi : 2 * c + oi + 1],
                        )

                # softmax denominators; fold 1/sum into the mix2 matrices
                m2b = {}
                for oi, ob in enumerate(("A", "B")):
                    sm = smx_pool.tile([128, 2], f32, tag=f"sm{ob}", name=f"sm{ob}")
                    nc.vector.reduce_sum(
                        sm[:, 0:1],
                        acc.rearrange("p (c o) -> p c o", o=2)[:, 0 : NKC + 1, oi],
                        axis=X,
                    )
                    rc = smx_pool.tile([128, 1], f32, tag=f"rc{ob}", name=f"rc{ob}")
                    nc.vector.reciprocal(rc[:], sm[:, 0:1])
                    m2b[ob] = smx_pool.tile(
                        [128, 256], bf16, tag=f"m2b{ob}", name=f"m2b{ob}"
                    )
                    nc.vector.tensor_scalar_mul(
                        m2b[ob][:], w2pt[f"w2p{ob}"][:], rc[:, 0:1]
                    )

                if pend is not None:
                    emit_mix2(*pend)
                pend = (qb_i, ex_s, m2b)

                # interleaved late work (PE: proj; DVE: next group's q~)
                if b == 0:
                    dma_q(1)
                    emit_qproj(1)
                if 1 <= b <= 4:
                    dma_v(b - 1)
                if 2 <= b <= 5:
                    emit_vproj(b - 2)
            emit_mix2(*pend)

            # build next group's q~ on the DVE while the PE runs A@V/fc
            # (this group's qtil has no readers left)
            if qg + 1 < NQG:
                qtil = [build_qtil(qg + 1, p) for p in range(4)]

            # ---- A@V for the group: attn^T[d, q] per out-head g ----
            # two av accumulation regions per PSUM bank: with bufs=2 the
            # copy-out of head g's result only gates head g+4's matmuls
            # (not g+2's), hiding the aT/at_s copy backlog on DVE/ACT
            at_s = []
            auxt = None
            for g in range(HEADS):
                if g % 2 == 0:
                    auxt = aux_ps.tile([128, 2 * QG], f32, tag="aux", name="av2")
                av = auxt[:, QG * (g % 2) : QG * (g % 2 + 1)]
                for j in range(NKJ):
                    nc.tensor.matmul(
                        av[0:HD, :],
                        lhsT=vS[j][:, HD * g : HD * (g + 1)],
                        rhs=aTv[:, j, g, :],
                        start=(j == 0),
                        stop=False,
                    )
                nc.tensor.matmul(
                    av[0:HD, :],
                    lhsT=vSp[:, HD * g : HD * (g + 1)],
                    rhs=aTpv[:, g, :],
                    start=False,
                    stop=True,
                )
                ats = at_pool.tile([HD, QG], f16, tag=f"ats{g}", name=f"ats{g}")
                eng = nc.scalar if g % 2 == 0 else nc.vector
                ecopy(eng, ats[:], av[0:HD, :])
                at_s.append(ats)

            # ---- fc_out: 8 accumulated K=64 matmuls + rank-1 bias ----
            # both 128-row output halves accumulate interleaved per head, so
            # the first fc matmul only waits on head 0's at_s copy (not 7's)
            fps = [
                m2_ps.tile([128, EMBED], f32, tag="m2", name=f"fp{sub}")
                for sub in (0, 1)
            ]
            for g in range(HEADS):
                for sub in (0, 1):
                    nc.tensor.matmul(
                        fps[sub][:],
                        lhsT=at_s[g][:, 128 * sub : 128 * (sub + 1)],
                        rhs=fcw[g][:],
                        start=(g == 0),
                        stop=False,
                    )
            for sub in (0, 1):
                nc.tensor.matmul(
                    fps[sub][:], lhsT=ones1[:], rhs=fcbt[:], start=False,
                    stop=True,
                )
                o_s = os_pool.tile([128, EMBED], f32, tag="os", name="o_s")
                nc.vector.tensor_copy(o_s[:], fps[sub][:])
                q_row = qg * QG + sub * 128
                nc.sync.dma_start(out_d[q_row : q_row + 128, :], o_s[:])

    nc.compile()
    return nc


def _get_nc():
    if "nc" not in _CACHED:
        _CACHED["nc"] = build_bass()
    return _CACHED["nc"]


def kernel(
    values,
    keys,
    queries,
    mask,
    Wv,
    Wk,
    Wq,
    W_pre,
    W_post,
    p_keys,
    p_values,
    fc_w,
    fc_b,
):
    """Full-input entry point. mask is all-True per the problem spec
    (fill: ones) and is therefore not consumed on-device."""
    from concourse.bass_utils import run_bass_kernel_spmd

    qT = np.asarray(queries, np.float32).astype(np.float16).transpose(0, 2, 1)
    kT = np.asarray(keys, np.float32).astype(np.float16).transpose(0, 2, 1)
    vT = np.asarray(values, np.float32).astype(np.float16).transpose(0, 2, 1)
    consts = _host_consts(
        np.asarray(Wv, np.float32),
        np.asarray(Wk, np.float32),
        np.asarray(Wq, np.float32),
        np.asarray(W_pre, np.float32),
        np.asarray(W_post, np.float32),
        np.asarray(p_keys, np.float32),
        np.asarray(p_values, np.float32),
        np.asarray(fc_w, np.float32),
        np.asarray(fc_b, np.float32),
    )

    nc = _get_nc()
    in_maps = []
    for core in range(N_CORES):
        n, half = core // 2, core % 2
        qbase = half * SQ
        m = {
            "qT16": np.ascontiguousarray(qT[n, :, qbase : qbase + SQ]),
            "kT16": np.ascontiguousarray(kT[n]),
            "vT16": np.ascontiguousarray(vT[n]),
            "dist": _dist_table(qbase),
        }
        m.update(consts)
        in_maps.append(m)

    global _last_in_maps
    _last_in_maps = in_maps
    res = run_bass_kernel_spmd(nc, in_maps, core_ids=list(range(N_CORES)))
    out = np.empty((N_BATCH, SEQ, EMBED), np.float32)
    for core in range(N_CORES):
        n, half = core // 2, core % 2
        out[n, half * SQ : (half + 1) * SQ, :] = res.results[core]["out"]
    return out
